# revision 9
# baseline (speedup 1.0000x reference)
"""ALIGN module kernel for 8 TRN2 NeuronCores (vocab-parallel).

Reference computation (B=4, S=576, Dv=1024, Dl=4096, V=32000):
    x  = vision_feats @ W1_w.T + W1_b          # [T=2304, Dl]
    xn = layernorm(x)                          # over Dl, no affine
    P  = softmax(xn @ W2_w.T, axis=-1)         # [T, V]
    F  = P @ llm_token_embed                   # [T, Dl]

Sharding: vocab dim of W2_w / llm_token_embed split across the 8 cores
(4000 rows each, zero-padded to 4096). Stage A (W1 + LN) is token-parallel
(288 tokens/core) followed by an AllGather of xn (bf16). Softmax needs no
max-subtraction (logits are ~N(0,1), |logit| < ~6): each core computes
exp(logits_loc); the 96 zero pad rows contribute exactly exp(0)=1 each, so
the local denominator is corrected by -96. Denominators are ReduceScattered
(each core only ever divides the 96-token slices it owns after the F
ReduceScatter, so no cross-core broadcast of s is needed at all).

Host-side prep encodes all layout work: weights arrive pre-transposed,
pre-padded, pre-tiled for unit-stride DMA, and pre-cast to bf16, so the
device never transposes W1/W2 and every big DMA runs at full burst size.
"""

import os
import sys

for _p in ("/opt/trn_rl_repo", "/root/.axon_site/_ro/trn_rl_repo"):
    if os.path.isdir(_p) and _p not in sys.path:
        sys.path.insert(0, _p)

import numpy as np
import ml_dtypes

from concourse import bass, bacc, mybir, tile
from concourse.bass_utils import run_bass_kernel_spmd
from concourse.masks import make_identity

BF16NP = ml_dtypes.bfloat16
F32 = mybir.dt.float32
BF16 = mybir.dt.bfloat16

N_CORES = 8
T = 2304          # total tokens (B*S)
T_LOC = 288       # tokens per core in stage A
DV = 1024
DL = 4096
V_PAD = 4096      # padded vocab rows per core (4000 real + 96 zero pads)
N_PAD = 96.0
NVT = V_PAD // 128  # 32 vocab tiles per core
NJ = DL // 128      # 32 contraction tiles

N_SB = 3          # token superblocks in phase B
TSB = 768         # tokens per superblock
N_TT = 6          # 128-row token tiles per superblock
C1 = 384          # matmul1 token-chunk width (2 chunks per superblock)
EC = 512          # matmul2 embedding-chunk width (SBUF tile)
N_EC = DL // EC   # 8 e-chunks
EG = 2            # e-chunks per ReduceScatter group
N_EG = N_EC // EG
EGW = EC * EG     # 1024 columns per RS
RS_ROWS = TSB // N_CORES  # 96 rows per core out of each ReduceScatter

_NC_CACHE = None


def build():
    nc = bacc.Bacc("TRN2", target_bir_lowering=False, debug=False,
                   num_devices=N_CORES)
    rg = [list(range(N_CORES))]

    visionT = nc.dram_tensor("visionT", [DV, T_LOC], BF16, kind="ExternalInput")
    w1t = nc.dram_tensor("w1t", [DV, DL], BF16, kind="ExternalInput")
    w1b = nc.dram_tensor("w1b", [1, DL], F32, kind="ExternalInput")
    # [vt][p][j][vi]: per-partition unit-stride 8KB runs
    w2t = nc.dram_tensor("w2t", [NVT, 128, NJ, 128], BF16, kind="ExternalInput")
    # [e][p][vt][n]: per-partition unit-stride 16KB runs
    emb = nc.dram_tensor("emb", [N_EC, 128, NVT, EC], BF16,
                         kind="ExternalInput")
    ones_v = nc.dram_tensor("ones_v", [128, NVT, 1], BF16, kind="ExternalInput")
    out = nc.dram_tensor("out", [N_SB, RS_ROWS, DL], F32, kind="ExternalOutput")

    from contextlib import ExitStack
    with tile.TileContext(nc) as tc, ExitStack() as ctx:
        consts = ctx.enter_context(tc.tile_pool(name="consts", bufs=1))
        dram = ctx.enter_context(tc.tile_pool(name="dram", bufs=1, space="DRAM"))
        dram_s = ctx.enter_context(tc.tile_pool(name="dram_s", bufs=2, space="DRAM"))
        dram_rs = ctx.enter_context(tc.tile_pool(name="dram_rs", bufs=3, space="DRAM"))
        if True:

            ident = consts.tile([128, 128], BF16)
            make_identity(nc, ident)
            onesv_sb = consts.tile([128, NVT, 1], BF16)
            nc.sync.dma_start(onesv_sb, ones_v[:])
            eps_col = consts.tile([128, 1], F32)
            nc.vector.memset(eps_col, 1e-5)

            ag_in = dram.tile([T_LOC, DL], BF16)
            # AllGather is split into the three stage-A token tiles so each
            # chunk's collective starts as soon as its LN finishes; output
            # chunk a holds rows (ta*c + r) for token 288c + 128a + r.
            ago0 = dram.tile([128 * N_CORES, DL], BF16, addr_space="Shared")
            ago1 = dram.tile([128 * N_CORES, DL], BF16, addr_space="Shared")
            ago2 = dram.tile([32 * N_CORES, DL], BF16, addr_space="Shared")
            ag_outs = [ago0, ago1, ago2]

            # ---------------- Stage A: x = visionT.T @ W1T + b, LN, -> bf16
            with ExitStack() as actx:
                sa = actx.enter_context(tc.tile_pool(name="stageA", bufs=1))
                sa2 = actx.enter_context(tc.tile_pool(name="stageA2", bufs=2))
                psa = actx.enter_context(tc.tile_pool(name="psumA", bufs=2, space="PSUM"))
                vt_sb = sa.tile([128, DV // 128, T_LOC], BF16)
                for k in range(DV // 128):
                    nc.sync.dma_start(
                        vt_sb[:, k, :], visionT[128 * k:128 * (k + 1), :])
                w1t_sb = sa.tile([128, DV // 128, DL], BF16)
                for k in range(DV // 128):
                    nc.sync.dma_start(
                        w1t_sb[:, k, :], w1t[128 * k:128 * (k + 1), :])
                bias_bc = sa.tile([128, DL], F32)
                nc.sync.dma_start(
                    bias_bc,
                    bass.AP(tensor=w1b, offset=0, ap=[[0, 128], [1, DL]]))

                t_sizes = [128, 128, 32]
                for a in range(3):
                    ta = t_sizes[a]
                    t0 = 128 * a
                    x_sb = sa2.tile([128, DL], F32, tag="x")
                    for n in range(DL // 512):
                        xp = psa.tile([128, 512], F32, tag="xp")
                        for k in range(DV // 128):
                            nc.tensor.matmul(
                                xp[:ta], lhsT=vt_sb[:, k, t0:t0 + ta],
                                rhs=w1t_sb[:, k, 512 * n:512 * (n + 1)],
                                start=(k == 0), stop=(k == DV // 128 - 1))
                        nc.vector.tensor_tensor(
                            out=x_sb[:ta, 512 * n:512 * (n + 1)],
                            in0=xp[:ta],
                            in1=bias_bc[:ta, 512 * n:512 * (n + 1)],
                            op=mybir.AluOpType.add)
                    # LayerNorm over DL
                    stats = sa2.tile([128, DL // 512, 6], F32, tag="stats")
                    for g in range(DL // 512):
                        nc.vector.bn_stats(
                            out=stats[:ta, g, :],
                            in_=x_sb[:ta, 512 * g:512 * (g + 1)])
                    mv = sa2.tile([128, 2], F32, tag="mv")
                    nc.vector.bn_aggr(out=mv[:ta], in_=stats[:ta])
                    sd = sa2.tile([128, 1], F32, tag="sd")
                    nc.scalar.activation(
                        out=sd[:ta], in_=mv[:ta, 1:2],
                        func=mybir.ActivationFunctionType.Sqrt,
                        bias=eps_col[:ta])
                    rstd = sa2.tile([128, 1], F32, tag="rstd")
                    nc.vector.reciprocal(out=rstd[:ta], in_=sd[:ta])
                    xn_bf = sa2.tile([128, DL], BF16, tag="xn")
                    nc.vector.tensor_scalar(
                        out=xn_bf[:ta], in0=x_sb[:ta],
                        scalar1=mv[:ta, 0:1], scalar2=rstd[:ta],
                        op0=mybir.AluOpType.subtract,
                        op1=mybir.AluOpType.mult)
                    nc.sync.dma_start(ag_in[t0:t0 + ta, :], xn_bf[:ta])

            for a, ta in enumerate((128, 128, 32)):
                nc.gpsimd.collective_compute(
                    "AllGather", mybir.AluOpType.bypass, replica_groups=rg,
                    ins=[ag_in[128 * a:128 * a + ta, :].opt()],
                    outs=[ag_outs[a].opt()])

            # ---------------- Phase B
            xnt_p = ctx.enter_context(tc.tile_pool(name="xnt_p", bufs=1))
            pt_p = ctx.enter_context(tc.tile_pool(name="pt_p", bufs=1))
            xl_p = ctx.enter_context(tc.tile_pool(name="xl_p", bufs=2))
            w2_p = ctx.enter_context(tc.tile_pool(name="w2_p", bufs=2))
            eb_p = ctx.enter_context(tc.tile_pool(name="eb_p", bufs=2))
            fs_p = ctx.enter_context(tc.tile_pool(name="fs_p", bufs=2))
            fo_p = ctx.enter_context(tc.tile_pool(name="fo_p", bufs=1))
            small = ctx.enter_context(tc.tile_pool(name="small", bufs=2))
            tp_ps = ctx.enter_context(tc.tile_pool(name="tp_ps", bufs=2, space="PSUM"))
            l_ps = ctx.enter_context(tc.tile_pool(name="l_ps", bufs=2, space="PSUM"))
            s_ps = ctx.enter_context(tc.tile_pool(name="s_ps", bufs=1, space="PSUM"))
            f_ps = ctx.enter_context(tc.tile_pool(name="f_ps", bufs=2, space="PSUM"))
            if True:

                for sb in range(N_SB):
                    # transpose xn superblock -> xnt [d_local, j, t_local]
                    xnt = xnt_p.tile([128, NJ, TSB], BF16, tag="xnt")
                    for tt in range(N_TT):
                        xl = xl_p.tile([128, DL], BF16, tag="xl")
                        g0 = TSB * sb + 128 * tt
                        g = g0
                        while g < g0 + 128:
                            c, rem = divmod(g, T_LOC)
                            a, r = divmod(rem, 128)
                            ta = 128 if a < 2 else 32
                            seg = min(g0 + 128 - g, ta - r, T_LOC - rem)
                            row = ta * c + r
                            nc.sync.dma_start(
                                xl[g - g0:g - g0 + seg, :],
                                ag_outs[a][row:row + seg, :])
                            g += seg
                        for j in range(NJ):
                            tp = tp_ps.tile([128, 128], BF16, tag="tp")
                            nc.tensor.transpose(
                                out=tp, in_=xl[:, 128 * j:128 * (j + 1)],
                                identity=ident)
                            nc.vector.tensor_copy(
                                out=xnt[:, j, 128 * tt:128 * (tt + 1)],
                                in_=tp)

                    # matmul1: logitsT per v-tile, exp -> pt
                    pt = pt_p.tile([128, NVT, TSB], BF16, tag="pt")
                    for vt in range(NVT):
                        w2s = w2_p.tile([128, NJ, 128], BF16, tag="w2")
                        nc.sync.dma_start(w2s, w2t[vt])
                        for c in range(2):
                            lp = l_ps.tile([128, C1], F32, tag="lp")
                            for j in range(NJ):
                                nc.tensor.matmul(
                                    lp, lhsT=w2s[:, j, :],
                                    rhs=xnt[:, j, C1 * c:C1 * (c + 1)],
                                    start=(j == 0), stop=(j == NJ - 1))
                            nc.scalar.activation(
                                out=pt[:, vt, C1 * c:C1 * (c + 1)], in_=lp,
                                func=mybir.ActivationFunctionType.Exp)

                    # denominator: s[t] = sum over real v rows of pt
                    # (onesv masks out the 96 zero-pad rows), then a tiny
                    # ReduceScatter hands each core exactly the 96-token
                    # slice it will own after the F ReduceScatters.
                    swidths = [(0, 512), (512, 256)]
                    sps = []
                    for c, (s0, sw) in enumerate(swidths):
                        sp = s_ps.tile([1, 512], F32, tag=f"sp{c}")
                        for vt in range(NVT):
                            nc.tensor.matmul(
                                sp[:, :sw], lhsT=onesv_sb[:, vt, :],
                                rhs=pt[:, vt, s0:s0 + sw],
                                start=(vt == 0), stop=(vt == NVT - 1))
                        sps.append(sp)
                    s_sb = small.tile([1, TSB], F32, tag="ssb", bufs=1)
                    for c, (s0, sw) in enumerate(swidths):
                        nc.vector.tensor_copy(
                            out=s_sb[0:1, s0:s0 + sw], in_=sps[c][:, :sw])
                    s_in = dram_s.tile([1, TSB], F32, tag="sin")
                    nc.sync.dma_start(s_in, s_sb)
                    s_out = dram_s.tile([1, RS_ROWS], F32, tag="sout")
                    nc.gpsimd.collective_compute(
                        "ReduceScatter", mybir.AluOpType.add, replica_groups=rg,
                        ins=[s_in.opt()], outs=[s_out.opt()])
                    sg = small.tile([RS_ROWS, 1], F32, tag="sg")
                    nc.sync.dma_start(
                        sg,
                        bass.AP(tensor=s_out.tensor, offset=s_out.offset,
                                ap=[[1, RS_ROWS], [1, 1]]))
                    rsg = small.tile([RS_ROWS, 1], F32, tag="rsg")
                    nc.vector.reciprocal(out=rsg, in_=sg)

                    # matmul2: F_partial = pt.T @ emb, RS per e-group, local
                    # divide on owned rows. Last superblock splits its final
                    # group to shrink the exposed RS tail.
                    egroups = [2, 2, 2, 2] if sb < N_SB - 1 else [2, 2, 2, 1, 1]
                    col = 0
                    e = 0
                    for gi, gsz in enumerate(egroups):
                        gw = gsz * EC
                        rs_in = dram_rs.tile([TSB, gw], F32, tag="rsin",
                                             name=f"rsin_{sb}_{gi}")
                        for ei in range(gsz):
                            eb = eb_p.tile([128, NVT, EC], BF16, tag="eb")
                            nc.sync.dma_start(eb, emb[e])
                            for tt in range(N_TT):
                                fp = f_ps.tile([128, EC], F32, tag="fp")
                                for vt in range(NVT):
                                    nc.tensor.matmul(
                                        fp,
                                        lhsT=pt[:, vt, 128 * tt:128 * (tt + 1)],
                                        rhs=eb[:, vt, :],
                                        start=(vt == 0), stop=(vt == NVT - 1))
                                fs = fs_p.tile([128, EC], F32, tag="fs")
                                nc.vector.tensor_copy(out=fs, in_=fp)
                                nc.sync.dma_start(
                                    rs_in[128 * tt:128 * (tt + 1),
                                          EC * ei:EC * (ei + 1)], fs)
                            e += 1
                        rs_out = dram_rs.tile([RS_ROWS, gw], F32, tag="rsout",
                                              name=f"rsout_{sb}_{gi}")
                        nc.gpsimd.collective_compute(
                            "ReduceScatter", mybir.AluOpType.add,
                            replica_groups=rg,
                            ins=[rs_in.opt()], outs=[rs_out.opt()])
                        fo = fo_p.tile([RS_ROWS, EGW], F32, tag="fo",
                                       name=f"fo_{sb}_{gi}")
                        nc.sync.dma_start(fo[:, :gw], rs_out[:])
                        fd = fo_p.tile([RS_ROWS, EGW], F32, tag="fd",
                                       name=f"fd_{sb}_{gi}")
                        nc.vector.tensor_scalar_mul(
                            out=fd[:, :gw], in0=fo[:, :gw], scalar1=rsg)
                        nc.sync.dma_start(
                            out[sb, :, col:col + gw], fd[:, :gw])
                        col += gw

    nc.compile()
    return nc


def _get_nc():
    global _NC_CACHE
    if _NC_CACHE is None:
        _NC_CACHE = build()
    return _NC_CACHE


def _prep_in_maps(vision_feats, W1_w, W1_b, W2_w, llm_token_embed):
    vf = np.ascontiguousarray(np.asarray(vision_feats, np.float32)).reshape(
        T, DV)
    W1 = np.asarray(W1_w, np.float32)
    b1 = np.ascontiguousarray(np.asarray(W1_b, np.float32)).reshape(1, DL)
    W2 = np.asarray(W2_w, np.float32)
    E = np.asarray(llm_token_embed, np.float32)

    w1t = np.ascontiguousarray(W1.T).astype(BF16NP)
    v_loc = 32000 // N_CORES
    in_maps = []
    for c in range(N_CORES):
        vT = np.ascontiguousarray(vf[T_LOC * c:T_LOC * (c + 1)].T).astype(
            BF16NP)
        w2p = np.zeros((V_PAD, DL), np.float32)
        w2p[:v_loc] = W2[v_loc * c:v_loc * (c + 1)]
        # [vt, p, j, vi] with p = d % 128, j = d // 128, vi = v % 128
        w2tt = w2p.T.reshape(NJ, 128, NVT, 128).transpose(2, 1, 0, 3).astype(
            BF16NP)
        ep = np.zeros((V_PAD, DL), np.float32)
        ep[:v_loc] = E[v_loc * c:v_loc * (c + 1)]
        # [e, p, vt, n] with p = v % 128, vt = v // 128, n = d % EC
        ebt = ep.reshape(NVT, 128, N_EC, EC).transpose(2, 1, 0, 3).astype(
            BF16NP)
        onesv = np.zeros((128, NVT, 1), np.float32)
        for vt in range(NVT):
            for p in range(128):
                if 128 * vt + p < v_loc:
                    onesv[p, vt, 0] = 1.0
        in_maps.append({
            "visionT": vT,
            "w1t": w1t,
            "w1b": b1,
            "w2t": np.ascontiguousarray(w2tt),
            "emb": np.ascontiguousarray(ebt),
            "ones_v": onesv.astype(BF16NP),
        })
    return in_maps


def run_on_cores(in_maps, trace=False, **kwargs):
    nc = _get_nc()
    return run_bass_kernel_spmd(nc, in_maps, core_ids=list(range(N_CORES)),
                                trace=trace, **kwargs)


def assemble(core_outs):
    full = np.empty((T, DL), np.float32)
    for c in range(N_CORES):
        o = np.asarray(core_outs[c])  # [N_SB, RS_ROWS, DL]
        for sb in range(N_SB):
            r0 = TSB * sb + RS_ROWS * c
            full[r0:r0 + RS_ROWS] = o[sb]
    return full.reshape(4, 576, DL)


def kernel(**inputs):
    in_maps = _prep_in_maps(**inputs)
    res = run_on_cores(in_maps)
    return assemble([r["out"] for r in res.results])


# revision 10
# speedup vs baseline: 1.0293x; 1.0293x over previous
"""ALIGN module kernel for 8 TRN2 NeuronCores (vocab-parallel).

Reference computation (B=4, S=576, Dv=1024, Dl=4096, V=32000):
    x  = vision_feats @ W1_w.T + W1_b          # [T=2304, Dl]
    xn = layernorm(x)                          # over Dl, no affine
    P  = softmax(xn @ W2_w.T, axis=-1)         # [T, V]
    F  = P @ llm_token_embed                   # [T, Dl]

Sharding: vocab dim of W2_w / llm_token_embed split across the 8 cores
(4000 rows each, zero-padded to 4096). Stage A (W1 + LN) is token-parallel
(288 tokens/core) followed by an AllGather of xn (bf16). Softmax needs no
max-subtraction (logits are ~N(0,1), |logit| < ~6): each core computes
exp(logits_loc); the 96 zero pad rows contribute exactly exp(0)=1 each, so
the local denominator is corrected by -96. Denominators are ReduceScattered
(each core only ever divides the 96-token slices it owns after the F
ReduceScatter, so no cross-core broadcast of s is needed at all).

Host-side prep encodes all layout work: weights arrive pre-transposed,
pre-padded, pre-tiled for unit-stride DMA, and pre-cast to bf16, so the
device never transposes W1/W2 and every big DMA runs at full burst size.
"""

import os
import sys

for _p in ("/opt/trn_rl_repo", "/root/.axon_site/_ro/trn_rl_repo"):
    if os.path.isdir(_p) and _p not in sys.path:
        sys.path.insert(0, _p)

import numpy as np
import ml_dtypes

from concourse import bass, bacc, mybir, tile
from concourse.bass_utils import run_bass_kernel_spmd
from concourse.masks import make_identity

BF16NP = ml_dtypes.bfloat16
F32 = mybir.dt.float32
BF16 = mybir.dt.bfloat16

N_CORES = 8
T = 2304          # total tokens (B*S)
T_LOC = 288       # tokens per core in stage A
DV = 1024
DL = 4096
V_PAD = 4096      # padded vocab rows per core (4000 real + 96 zero pads)
N_PAD = 96.0
NVT = V_PAD // 128  # 32 vocab tiles per core
NJ = DL // 128      # 32 contraction tiles

N_SB = 3          # token superblocks in phase B
TSB = 768         # tokens per superblock
N_TT = 6          # 128-row token tiles per superblock
C1 = 384          # matmul1 token-chunk width (2 chunks per superblock)
EC = 512          # matmul2 embedding-chunk width (SBUF tile)
N_EC = DL // EC   # 8 e-chunks
EG = 2            # e-chunks per ReduceScatter group
N_EG = N_EC // EG
EGW = EC * EG     # 1024 columns per RS
RS_ROWS = TSB // N_CORES  # 96 rows per core out of each ReduceScatter

_NC_CACHE = None


def build():
    nc = bacc.Bacc("TRN2", target_bir_lowering=False, debug=False,
                   num_devices=N_CORES)
    rg = [list(range(N_CORES))]

    visionT = nc.dram_tensor("visionT", [DV, T_LOC], BF16, kind="ExternalInput")
    w1t = nc.dram_tensor("w1t", [DV, DL], BF16, kind="ExternalInput")
    w1b = nc.dram_tensor("w1b", [1, DL], F32, kind="ExternalInput")
    # [vt][p][j][vi]: per-partition unit-stride 8KB runs
    w2t = nc.dram_tensor("w2t", [NVT, 128, NJ, 128], BF16, kind="ExternalInput")
    # [e][p][vt][n]: per-partition unit-stride 16KB runs
    emb = nc.dram_tensor("emb", [N_EC, 128, NVT, EC], BF16,
                         kind="ExternalInput")
    ones_v = nc.dram_tensor("ones_v", [128, NVT, 1], BF16, kind="ExternalInput")
    out = nc.dram_tensor("out", [N_SB, RS_ROWS, DL], F32, kind="ExternalOutput")

    from contextlib import ExitStack
    with tile.TileContext(nc) as tc, ExitStack() as ctx:
        consts = ctx.enter_context(tc.tile_pool(name="consts", bufs=1))
        dram = ctx.enter_context(tc.tile_pool(name="dram", bufs=1, space="DRAM"))
        dram_s = ctx.enter_context(tc.tile_pool(name="dram_s", bufs=2, space="DRAM"))
        dram_rs = ctx.enter_context(tc.tile_pool(name="dram_rs", bufs=3, space="DRAM"))
        if True:

            ident = consts.tile([128, 128], BF16)
            make_identity(nc, ident)
            onesv_sb = consts.tile([128, NVT, 1], BF16)
            nc.sync.dma_start(onesv_sb, ones_v[:])
            eps_col = consts.tile([128, 1], F32)
            nc.vector.memset(eps_col, 1e-5)

            ag_in = dram.tile([T_LOC, DL], BF16)
            ag_out = dram.tile([T, DL], BF16, addr_space="Shared")

            # ---------------- Stage A: x = visionT.T @ W1T + b, LN, -> bf16
            with ExitStack() as actx:
                sa = actx.enter_context(tc.tile_pool(name="stageA", bufs=1))
                sa2 = actx.enter_context(tc.tile_pool(name="stageA2", bufs=2))
                psa = actx.enter_context(tc.tile_pool(name="psumA", bufs=2, space="PSUM"))
                vt_sb = sa.tile([128, DV // 128, T_LOC], BF16)
                for k in range(DV // 128):
                    nc.sync.dma_start(
                        vt_sb[:, k, :], visionT[128 * k:128 * (k + 1), :])
                w1t_sb = sa.tile([128, DV // 128, DL], BF16)
                for k in range(DV // 128):
                    nc.sync.dma_start(
                        w1t_sb[:, k, :], w1t[128 * k:128 * (k + 1), :])
                bias_bc = sa.tile([128, DL], F32)
                nc.sync.dma_start(
                    bias_bc,
                    bass.AP(tensor=w1b, offset=0, ap=[[0, 128], [1, DL]]))

                t_sizes = [128, 128, 32]
                for a in range(3):
                    ta = t_sizes[a]
                    t0 = 128 * a
                    x_sb = sa2.tile([128, DL], F32, tag="x")
                    for n in range(DL // 512):
                        xp = psa.tile([128, 512], F32, tag="xp")
                        for k in range(DV // 128):
                            nc.tensor.matmul(
                                xp[:ta], lhsT=vt_sb[:, k, t0:t0 + ta],
                                rhs=w1t_sb[:, k, 512 * n:512 * (n + 1)],
                                start=(k == 0), stop=(k == DV // 128 - 1))
                        nc.vector.tensor_tensor(
                            out=x_sb[:ta, 512 * n:512 * (n + 1)],
                            in0=xp[:ta],
                            in1=bias_bc[:ta, 512 * n:512 * (n + 1)],
                            op=mybir.AluOpType.add)
                    # LayerNorm over DL
                    stats = sa2.tile([128, DL // 512, 6], F32, tag="stats")
                    for g in range(DL // 512):
                        nc.vector.bn_stats(
                            out=stats[:ta, g, :],
                            in_=x_sb[:ta, 512 * g:512 * (g + 1)])
                    mv = sa2.tile([128, 2], F32, tag="mv")
                    nc.vector.bn_aggr(out=mv[:ta], in_=stats[:ta])
                    sd = sa2.tile([128, 1], F32, tag="sd")
                    nc.scalar.activation(
                        out=sd[:ta], in_=mv[:ta, 1:2],
                        func=mybir.ActivationFunctionType.Sqrt,
                        bias=eps_col[:ta])
                    rstd = sa2.tile([128, 1], F32, tag="rstd")
                    nc.vector.reciprocal(out=rstd[:ta], in_=sd[:ta])
                    xn_bf = sa2.tile([128, DL], BF16, tag="xn")
                    nc.vector.tensor_scalar(
                        out=xn_bf[:ta], in0=x_sb[:ta],
                        scalar1=mv[:ta, 0:1], scalar2=rstd[:ta],
                        op0=mybir.AluOpType.subtract,
                        op1=mybir.AluOpType.mult)
                    nc.sync.dma_start(ag_in[t0:t0 + ta, :], xn_bf[:ta])

            nc.gpsimd.collective_compute(
                "AllGather", mybir.AluOpType.bypass, replica_groups=rg,
                ins=[ag_in.opt()], outs=[ag_out.opt()])

            # ---------------- Phase B
            xnt_p = ctx.enter_context(tc.tile_pool(name="xnt_p", bufs=1))
            pt_p = ctx.enter_context(tc.tile_pool(name="pt_p", bufs=1))
            xl_p = ctx.enter_context(tc.tile_pool(name="xl_p", bufs=2))
            w2_p = ctx.enter_context(tc.tile_pool(name="w2_p", bufs=2))
            eb_p = ctx.enter_context(tc.tile_pool(name="eb_p", bufs=2))
            fs_p = ctx.enter_context(tc.tile_pool(name="fs_p", bufs=2))
            fo_p = ctx.enter_context(tc.tile_pool(name="fo_p", bufs=1))
            small = ctx.enter_context(tc.tile_pool(name="small", bufs=2))
            tp_ps = ctx.enter_context(tc.tile_pool(name="tp_ps", bufs=2, space="PSUM"))
            l_ps = ctx.enter_context(tc.tile_pool(name="l_ps", bufs=2, space="PSUM"))
            s_ps = ctx.enter_context(tc.tile_pool(name="s_ps", bufs=1, space="PSUM"))
            f_ps = ctx.enter_context(tc.tile_pool(name="f_ps", bufs=2, space="PSUM"))
            if True:

                def make_xnt(sb):
                    # transpose xn superblock -> xnt [d_local, j, t_local]
                    xnt = xnt_p.tile([128, NJ, TSB], BF16, tag="xnt",
                                     name=f"xnt_{sb}")
                    for tt in range(N_TT):
                        xl = xl_p.tile([128, DL], BF16, tag="xl",
                                       name=f"xl_{sb}_{tt}")
                        r0 = TSB * sb + 128 * tt
                        nc.sync.dma_start(xl, ag_out[r0:r0 + 128, :])
                        for j in range(NJ):
                            tp = tp_ps.tile([128, 128], BF16, tag="tp",
                                            name=f"tp_{sb}_{tt}_{j}")
                            nc.tensor.transpose(
                                out=tp, in_=xl[:, 128 * j:128 * (j + 1)],
                                identity=ident)
                            nc.vector.tensor_copy(
                                out=xnt[:, j, 128 * tt:128 * (tt + 1)],
                                in_=tp)
                    return xnt

                xnt = make_xnt(0)
                for sb in range(N_SB):
                    # matmul1: logitsT per v-tile, exp -> pt
                    pt = pt_p.tile([128, NVT, TSB], BF16, tag="pt")
                    for vt in range(NVT):
                        w2s = w2_p.tile([128, NJ, 128], BF16, tag="w2")
                        nc.sync.dma_start(w2s, w2t[vt])
                        for c in range(2):
                            lp = l_ps.tile([128, C1], F32, tag="lp")
                            for j in range(NJ):
                                nc.tensor.matmul(
                                    lp, lhsT=w2s[:, j, :],
                                    rhs=xnt[:, j, C1 * c:C1 * (c + 1)],
                                    start=(j == 0), stop=(j == NJ - 1))
                            nc.scalar.activation(
                                out=pt[:, vt, C1 * c:C1 * (c + 1)], in_=lp,
                                func=mybir.ActivationFunctionType.Exp)

                    # queue next superblock's transposes ahead of matmul2
                    # so the scheduler overlaps them with this superblock
                    next_xnt = make_xnt(sb + 1) if sb + 1 < N_SB else None

                    # denominator: s[t] = sum over real v rows of pt
                    # (onesv masks out the 96 zero-pad rows), then a tiny
                    # ReduceScatter hands each core exactly the 96-token
                    # slice it will own after the F ReduceScatters.
                    swidths = [(0, 512), (512, 256)]
                    sps = []
                    for c, (s0, sw) in enumerate(swidths):
                        sp = s_ps.tile([1, 512], F32, tag=f"sp{c}")
                        for vt in range(NVT):
                            nc.tensor.matmul(
                                sp[:, :sw], lhsT=onesv_sb[:, vt, :],
                                rhs=pt[:, vt, s0:s0 + sw],
                                start=(vt == 0), stop=(vt == NVT - 1))
                        sps.append(sp)
                    s_sb = small.tile([1, TSB], F32, tag="ssb", bufs=1)
                    for c, (s0, sw) in enumerate(swidths):
                        nc.vector.tensor_copy(
                            out=s_sb[0:1, s0:s0 + sw], in_=sps[c][:, :sw])
                    s_in = dram_s.tile([1, TSB], F32, tag="sin")
                    nc.sync.dma_start(s_in, s_sb)
                    s_out = dram_s.tile([1, RS_ROWS], F32, tag="sout")
                    nc.gpsimd.collective_compute(
                        "ReduceScatter", mybir.AluOpType.add, replica_groups=rg,
                        ins=[s_in.opt()], outs=[s_out.opt()])
                    sg = small.tile([RS_ROWS, 1], F32, tag="sg")
                    nc.sync.dma_start(
                        sg,
                        bass.AP(tensor=s_out.tensor, offset=s_out.offset,
                                ap=[[1, RS_ROWS], [1, 1]]))
                    rsg = small.tile([RS_ROWS, 1], F32, tag="rsg")
                    nc.vector.reciprocal(out=rsg, in_=sg)

                    # matmul2: F_partial = pt.T @ emb, RS per e-group, local
                    # divide on owned rows. Last superblock splits its final
                    # group to shrink the exposed RS tail.
                    egroups = [2, 2, 2, 2] if sb < N_SB - 1 else [2, 2, 2, 1, 1]
                    col = 0
                    e = 0
                    for gi, gsz in enumerate(egroups):
                        gw = gsz * EC
                        rs_in = dram_rs.tile([TSB, gw], F32, tag="rsin",
                                             name=f"rsin_{sb}_{gi}")
                        for ei in range(gsz):
                            eb = eb_p.tile([128, NVT, EC], BF16, tag="eb")
                            nc.sync.dma_start(eb, emb[e])
                            for tt in range(N_TT):
                                fp = f_ps.tile([128, EC], F32, tag="fp")
                                for vt in range(NVT):
                                    nc.tensor.matmul(
                                        fp,
                                        lhsT=pt[:, vt, 128 * tt:128 * (tt + 1)],
                                        rhs=eb[:, vt, :],
                                        start=(vt == 0), stop=(vt == NVT - 1))
                                fs = fs_p.tile([128, EC], F32, tag="fs")
                                nc.vector.tensor_copy(out=fs, in_=fp)
                                nc.sync.dma_start(
                                    rs_in[128 * tt:128 * (tt + 1),
                                          EC * ei:EC * (ei + 1)], fs)
                            e += 1
                        rs_out = dram_rs.tile([RS_ROWS, gw], F32, tag="rsout",
                                              name=f"rsout_{sb}_{gi}")
                        nc.gpsimd.collective_compute(
                            "ReduceScatter", mybir.AluOpType.add,
                            replica_groups=rg,
                            ins=[rs_in.opt()], outs=[rs_out.opt()])
                        fo = fo_p.tile([RS_ROWS, EGW], F32, tag="fo",
                                       name=f"fo_{sb}_{gi}")
                        nc.sync.dma_start(fo[:, :gw], rs_out[:])
                        fd = fo_p.tile([RS_ROWS, EGW], F32, tag="fd",
                                       name=f"fd_{sb}_{gi}")
                        nc.vector.tensor_scalar_mul(
                            out=fd[:, :gw], in0=fo[:, :gw], scalar1=rsg)
                        nc.sync.dma_start(
                            out[sb, :, col:col + gw], fd[:, :gw])
                        col += gw
                    xnt = next_xnt

    nc.compile()
    return nc


def _get_nc():
    global _NC_CACHE
    if _NC_CACHE is None:
        _NC_CACHE = build()
    return _NC_CACHE


def _prep_in_maps(vision_feats, W1_w, W1_b, W2_w, llm_token_embed):
    vf = np.ascontiguousarray(np.asarray(vision_feats, np.float32)).reshape(
        T, DV)
    W1 = np.asarray(W1_w, np.float32)
    b1 = np.ascontiguousarray(np.asarray(W1_b, np.float32)).reshape(1, DL)
    W2 = np.asarray(W2_w, np.float32)
    E = np.asarray(llm_token_embed, np.float32)

    w1t = np.ascontiguousarray(W1.T).astype(BF16NP)
    v_loc = 32000 // N_CORES
    in_maps = []
    for c in range(N_CORES):
        vT = np.ascontiguousarray(vf[T_LOC * c:T_LOC * (c + 1)].T).astype(
            BF16NP)
        w2p = np.zeros((V_PAD, DL), np.float32)
        w2p[:v_loc] = W2[v_loc * c:v_loc * (c + 1)]
        # [vt, p, j, vi] with p = d % 128, j = d // 128, vi = v % 128
        w2tt = w2p.T.reshape(NJ, 128, NVT, 128).transpose(2, 1, 0, 3).astype(
            BF16NP)
        ep = np.zeros((V_PAD, DL), np.float32)
        ep[:v_loc] = E[v_loc * c:v_loc * (c + 1)]
        # [e, p, vt, n] with p = v % 128, vt = v // 128, n = d % EC
        ebt = ep.reshape(NVT, 128, N_EC, EC).transpose(2, 1, 0, 3).astype(
            BF16NP)
        onesv = np.zeros((128, NVT, 1), np.float32)
        for vt in range(NVT):
            for p in range(128):
                if 128 * vt + p < v_loc:
                    onesv[p, vt, 0] = 1.0
        in_maps.append({
            "visionT": vT,
            "w1t": w1t,
            "w1b": b1,
            "w2t": np.ascontiguousarray(w2tt),
            "emb": np.ascontiguousarray(ebt),
            "ones_v": onesv.astype(BF16NP),
        })
    return in_maps


def run_on_cores(in_maps, trace=False, **kwargs):
    nc = _get_nc()
    return run_bass_kernel_spmd(nc, in_maps, core_ids=list(range(N_CORES)),
                                trace=trace, **kwargs)


def assemble(core_outs):
    full = np.empty((T, DL), np.float32)
    for c in range(N_CORES):
        o = np.asarray(core_outs[c])  # [N_SB, RS_ROWS, DL]
        for sb in range(N_SB):
            r0 = TSB * sb + RS_ROWS * c
            full[r0:r0 + RS_ROWS] = o[sb]
    return full.reshape(4, 576, DL)


def kernel(**inputs):
    in_maps = _prep_in_maps(**inputs)
    res = run_on_cores(in_maps)
    return assemble([r["out"] for r in res.results])
